# revision 18
# baseline (speedup 1.0000x reference)
"""Trainium2 Bass kernel for nn_CNN_9818295238933 (gnn_message_passing).

Pipeline per sample (data-parallel over batch across 8 cores):
  conv1 (einsum bcfn,kcn->bkf) -> h1 table [F,K] rows in HBM
  gather(adj) via indirect DMA (128B rows) -> PE transpose -> conv2 -> h2 table
  gather(adj) -> PE transpose -> conv3 -> flat bounce
AllToAll redistributes flat so each core owns a contraction chunk of fc1;
partial y1 accumulated in PSUM, AllReduced; BN+ReLU+fc2+BN+ReLU+fco replicated.

Self-contained: hardcodes all shapes; only imports the Trainium toolchain.
"""

import sys
from dataclasses import dataclass, field

if "/opt/trn_rl_repo" not in sys.path:
    sys.path.insert(0, "/opt/trn_rl_repo")

import numpy as np


@dataclass(frozen=True)
class Cfg:
    ncores: int = 8
    B: int = 64
    C: int = 12
    N: int = 7
    K: int = 32
    F: int = 9000
    F_pad: int = 9216
    CH: int = 128
    H1: int = 100
    H2: int = 30
    NCLS: int = 2
    EPS: float = 1e-5

    @property
    def BL(self):
        return self.B // self.ncores

    @property
    def CN(self):
        return self.C * self.N

    @property
    def NCH(self):
        return self.F_pad // self.CH

    @property
    def KL(self):
        return self.K // self.ncores

    @property
    def NA(self):
        return 4

    @property
    def NB(self):
        return self.N - 4


CFG = Cfg()


# ---------------------------------------------------------------------------
# Host-side input preparation
# ---------------------------------------------------------------------------

def prep_core_inputs(cfg: Cfg, x, adjacencies, W1, W2, W3, fc1_w, fc1_b, bn1_g,
                     bn1_b, fc2_w, fc2_b, bn2_g, bn2_b, fco_w, fco_b):
    """Build the per-core input maps (list of dicts, one per core)."""
    B, C, N, K, F, FP, CH = (cfg.B, cfg.C, cfg.N, cfg.K, cfg.F, cfg.F_pad,
                             cfg.CH)
    BL, CN, NCH, KL, NA, NB = cfg.BL, cfg.CN, cfg.NCH, cfg.KL, cfg.NA, cfg.NB
    H1, H2, NCLS = cfg.H1, cfg.H2, cfg.NCLS

    x = np.asarray(x, dtype=np.float32)
    adj = np.asarray(adjacencies).astype(np.int64)[:, 0]  # [B, F, N]

    # x [B, C, F, N] -> xt [B, (c,n), F_pad] fp16, zero-padded along f.
    xt = np.zeros((B, CN, FP), dtype=np.float16)
    xt[:, :, :F] = np.transpose(x, (0, 1, 3, 2)).reshape(B, CN, F)

    # Index tensor for dma_gather: entry e = (ch*N + n)*CH + p gathers table
    # row adj[b, ch*CH+p, n]; int16 wrapped across 16 partitions
    # (entry e at [e%16, e//16]).  Chunk ch runs on SWDGE queue ch%4, whose
    # Q7 core pair reads indices from partitions 32q..32q+31 only, so the
    # packed layout stores chunk 4j+q's wrap at rows 32q..32q+31 (copied to
    # both 16-row halves), column block j.
    assert CH == 128, "dma_gather entry->partition mapping requires CH=128"
    adj_pad = np.zeros((B, FP, N), dtype=np.int64)
    adj_pad[:, :F] = adj
    adj_r = adj_pad.reshape(B, NCH, CH, N)

    w1f = np.transpose(np.asarray(W1, np.float16), (1, 2, 0)).reshape(CN, K)

    def stack(Wm):  # [K_out, K_in, N] -> A [NA*K_in, K_out], B [NB*K_in, K_out]
        Wm = np.asarray(Wm, np.float32)
        wa = np.transpose(Wm[:, :, :NA], (2, 1, 0)).reshape(NA * K, K)
        wb = np.transpose(Wm[:, :, NA:], (2, 1, 0)).reshape(NB * K, K)
        return np.ascontiguousarray(wa), np.ascontiguousarray(wb)

    w2a, w2b = stack(W2)
    w3a, w3b = stack(W3)

    # fc1 weights: [H1, K*F] -> padded-transposed [K, F_pad, H1] fp16
    fc1 = np.asarray(fc1_w, np.float32).reshape(H1, K, F)
    fc1t = np.zeros((K, FP, H1), dtype=np.float16)
    fc1t[:, :F] = np.transpose(fc1, (1, 2, 0))

    fc2wt = np.ascontiguousarray(np.asarray(fc2_w, np.float32).T)  # [H1, H2]
    fcowt = np.ascontiguousarray(np.asarray(fco_w, np.float32).T)  # [H2, NCLS]

    def col(v, n):
        return np.asarray(v, np.float32).reshape(n, 1)

    shared = dict(
        w1=w1f, w2a=w2a, w2b=w2b, w3a=w3a, w3b=w3b,
        fc1b=col(fc1_b, H1), bn1g=col(bn1_g, H1), bn1b=col(bn1_b, H1),
        fc2wt=fc2wt, fc2b=col(fc2_b, H2), bn2g=col(bn2_g, H2),
        bn2b=col(bn2_b, H2), fcowt=fcowt, fcob=col(fco_b, NCLS),
    )

    NQ = 4
    JC = NCH // NQ                   # column blocks (local chunks per queue)
    WC = N * CH // 16                # wrapped columns per chunk (56)
    in_maps = []
    for c in range(cfg.ncores):
        bsl = slice(c * BL, (c + 1) * BL)
        idx = adj_r[bsl]  # [BL, NCH, CH, N]
        # per-chunk entry order (n, p), wrapped 16:
        # wrap [BL, NCH, 16, WC]
        wrap = np.transpose(idx, (0, 1, 3, 2)).reshape(
            BL, NCH, WC, 16).transpose(0, 1, 3, 2).astype(np.int16)
        i16 = np.zeros((BL, 128, JC * WC), dtype=np.int16)
        for q in range(NQ):
            for j in range(JC):
                w = wrap[:, j * NQ + q]                  # [BL, 16, WC]
                i16[:, 32 * q:32 * q + 16, j * WC:(j + 1) * WC] = w
                i16[:, 32 * q + 16:32 * q + 32, j * WC:(j + 1) * WC] = w
        fc1wt_c = np.ascontiguousarray(
            fc1t[c * KL:(c + 1) * KL].reshape(KL * FP, H1))
        m = dict(shared)
        m.update(
            xt=np.ascontiguousarray(xt[bsl]),
            idx16=np.ascontiguousarray(i16), fc1wt=fc1wt_c,
        )
        in_maps.append(m)
    return in_maps


# ---------------------------------------------------------------------------
# Device program
# ---------------------------------------------------------------------------

def build_program(cfg: Cfg, dbg: bool = False, reps: int = 1,
                  probe: str = "", sect: str = "pipe"):
    """probe: timing-only variants ('nogather' memsets instead of dma_gather,
    'gatheronly' skips the per-chunk compute, 'noconv1' skips conv1).
    sect: which section `reps` repeats ('pipe' = conv/gather/A2A loop,
    'fc1' = fc1 contraction stage, 'a2a' = AllToAll only)."""
    import concourse.bass as bass
    import concourse.bacc as bacc
    import concourse.mybir as mybir
    import concourse.tile as tile
    from concourse.masks import make_identity

    dt = mybir.dt.float32
    dth = mybir.dt.float16
    B, C, N, K, FP, CH = cfg.B, cfg.C, cfg.N, cfg.K, cfg.F_pad, cfg.CH
    BL, CN, NCH, KL, NA, NB = cfg.BL, cfg.CN, cfg.NCH, cfg.KL, cfg.NA, cfg.NB
    H1, H2, NCLS = cfg.H1, cfg.H2, cfg.NCLS
    NCORES = cfg.ncores
    rg = [list(range(NCORES))]

    nc = bacc.Bacc("TRN2", target_bir_lowering=False, debug=False,
                   num_devices=NCORES, num_swdge_queues=4)

    TW = 64                          # table row width (256 B granularity)
    i16t = mybir.dt.int16
    WC = N * CH // 16                # wrapped idx cols per chunk (56)
    LIDX = (NCH // 4) * WC           # per-queue packed idx cols (1008)

    xt = nc.dram_tensor("xt", [BL, CN, FP], dth, kind="ExternalInput")
    idx16 = nc.dram_tensor("idx16", [BL, 128, LIDX], i16t,
                           kind="ExternalInput")
    w1 = nc.dram_tensor("w1", [CN, K], dth, kind="ExternalInput")
    w2a = nc.dram_tensor("w2a", [NA * K, K], dt, kind="ExternalInput")
    w2b = nc.dram_tensor("w2b", [NB * K, K], dt, kind="ExternalInput")
    w3a = nc.dram_tensor("w3a", [NA * K, K], dt, kind="ExternalInput")
    w3b = nc.dram_tensor("w3b", [NB * K, K], dt, kind="ExternalInput")
    fc1wt = nc.dram_tensor("fc1wt", [KL * FP, H1], dth, kind="ExternalInput")
    fc1b = nc.dram_tensor("fc1b", [H1, 1], dt, kind="ExternalInput")
    bn1g = nc.dram_tensor("bn1g", [H1, 1], dt, kind="ExternalInput")
    bn1b = nc.dram_tensor("bn1b", [H1, 1], dt, kind="ExternalInput")
    fc2wt = nc.dram_tensor("fc2wt", [H1, H2], dt, kind="ExternalInput")
    fc2b = nc.dram_tensor("fc2b", [H2, 1], dt, kind="ExternalInput")
    bn2g = nc.dram_tensor("bn2g", [H2, 1], dt, kind="ExternalInput")
    bn2b = nc.dram_tensor("bn2b", [H2, 1], dt, kind="ExternalInput")
    fcowt = nc.dram_tensor("fcowt", [H2, NCLS], dt, kind="ExternalInput")
    fcob = nc.dram_tensor("fcob", [NCLS, 1], dt, kind="ExternalInput")
    out = nc.dram_tensor("out", [NCLS, B], dt, kind="ExternalOutput")
    if dbg:
        dbg_h1 = nc.dram_tensor("dbg_h1", [FP, TW], dt, kind="ExternalOutput")
        dbg_h2 = nc.dram_tensor("dbg_h2", [FP, TW], dt, kind="ExternalOutput")
        dbg_ga = nc.dram_tensor("dbg_ga", [CH, N * TW], dt,
                                kind="ExternalOutput")
        dbg_bnc = nc.dram_tensor("dbg_bnc", [NCORES, KL, BL, FP], dt,
                                 kind="ExternalOutput")
        dbg_rcv = nc.dram_tensor("dbg_rcv", [NCORES, KL, BL, FP], dt,
                                 kind="ExternalOutput")
        dbg_y1 = nc.dram_tensor("dbg_y1", [H1, B], dt, kind="ExternalOutput")

    with tile.TileContext(nc) as tc:
        with (
            tc.tile_pool(name="consts", bufs=1) as consts,
            tc.tile_pool(name="xpool", bufs=2) as xpool,
            tc.tile_pool(name="gpool", bufs=2) as gpool,
            tc.tile_pool(name="idxp", bufs=2) as idxp,
            tc.tile_pool(name="work", bufs=3) as work,
            tc.tile_pool(name="dram", bufs=1, space="DRAM") as dram,
        ):
            # ---- constants ----
            ident = consts.tile([CH, CH], dt)
            make_identity(nc, ident)
            zcol = consts.tile([CH, 1], dt)
            nc.vector.memset(zcol[:], 0.0)
            w1_t = consts.tile([CN, K], dth)
            nc.sync.dma_start(w1_t[:], w1[:])
            w2a_t = consts.tile([NA * K, K], dt)
            nc.sync.dma_start(w2a_t[:], w2a[:])
            w2b_t = consts.tile([NB * K, K], dt)
            nc.sync.dma_start(w2b_t[:], w2b[:])
            w3a_t = consts.tile([NA * K, K], dt)
            nc.sync.dma_start(w3a_t[:], w3a[:])
            w3b_t = consts.tile([NB * K, K], dt)
            nc.sync.dma_start(w3b_t[:], w3b[:])

            # ---- DRAM scratch (per-sample gather tables, 256B rows) ----
            htab1 = [dram.tile([FP, TW], dt, name=f"htab1_{b}")
                     for b in range(BL)]
            htab2 = [dram.tile([FP, TW], dt, name=f"htab2_{b}")
                     for b in range(BL)]
            bounce = dram.tile([NCORES, KL, BL, FP], dt)
            recv = dram.tile([NCORES, KL, BL, FP], dt)
            y1snd = dram.tile([H1, B], dt)
            y1rcv = dram.tile([H1, B], dt)

            with tc.tile_pool(name="cpsum", bufs=1, space="PSUM") as cpsum:

                def tab_write(tab, hall):
                    """Single DMA: SBUF [CH, NCH*K] -> table rows (ch*CH+p).
                    Only the first K of each TW-wide row is written; the pad
                    columns are never read by compute."""
                    nc.sync.dma_start(
                        tab[:, 0:K].rearrange("(ch p) k -> p ch k", p=CH),
                        hall.rearrange("p (ch k) -> p ch k", k=K))

                def conv1(b):
                    xtile = xpool.tile([CN, FP], dth, tag="xt")
                    nc.sync.dma_start(xtile[:], xt[b])
                    hall = work.tile([CH, NCH * K], dt, tag="hall", bufs=2)
                    for ch in range(NCH):
                        ps = cpsum.tile([CH, K], dt, tag="cK", bufs=2)
                        nc.tensor.matmul(
                            out=ps[:], lhsT=xtile[:, ch * CH:(ch + 1) * CH],
                            rhs=w1_t[:], start=True, stop=True)
                        nc.vector.tensor_copy(hall[:, ch * K:(ch + 1) * K],
                                              ps[:])
                    tab_write(htab1[b][:], hall[:])

                NIC = N * CH  # indices per chunk-gather (896 <= 1024 limit)
                r_nic = nc.gpsimd.to_reg(NIC)

                def glayer(b, src_tab, wa_t, wb_t, sink, capture_ga=False):
                    """Gather from src_tab with this sample's adjacency, then
                    conv with stacked weights; sink(ch, gta, gtb) consumes the
                    transposed gather tiles."""
                    it = idxp.tile([128, LIDX], i16t, tag="it")
                    nc.sync.dma_start(it[:], idx16[b])
                    for ch in range(NCH):
                        g = gpool.tile([CH, N, TW], dt, tag="g", bufs=8)
                        if probe == "seqdma":
                            nc.sync.dma_start(
                                g[:].rearrange("p s e -> p (s e)"),
                                src_tab[0:N * CH, :])
                        elif probe in ("nogather", "nognoc"):
                            nc.vector.memset(
                                g[:].rearrange("p s e -> p (s e)"), 0.0)
                        else:
                            nc.gpsimd.dma_gather(
                                out_ap=g[:], in_ap=src_tab[:],
                                idxs_ap=it[:, (ch // 4) * WC:
                                           (ch // 4 + 1) * WC],
                                num_idxs=NIC, num_idxs_reg=r_nic,
                                elem_size=TW, queue_num=ch % 4)
                        if capture_ga and ch == 0:
                            nc.sync.dma_start(
                                dbg_ga[:], g[:].rearrange("p s e -> p (s e)"))
                        if probe in ("gatheronly", "nognoc"):
                            continue
                        gpack = work.tile([CH, N * K], dt, tag="gpack")
                        nc.vector.tensor_copy(
                            gpack[:].rearrange("p (s k) -> p s k", k=K),
                            g[:, 0:N, 0:K])
                        psa = cpsum.tile([NA * K, CH], dt, tag="tT", bufs=3)
                        nc.tensor.transpose(
                            psa[:], gpack[:, 0:NA * K], ident[:])
                        gta = work.tile([NA * K, CH], dt, tag="gta")
                        nc.vector.tensor_copy(gta[:], psa[:])
                        psb = cpsum.tile([NB * K, CH], dt, tag="tT", bufs=3)
                        nc.tensor.transpose(
                            psb[:], gpack[:, NA * K:N * K], ident[:])
                        gtb = work.tile([NB * K, CH], dt, tag="gtb")
                        nc.vector.tensor_copy(gtb[:], psb[:])
                        sink(ch, gta, gtb)

                def conv2_sink(b):
                    hall = work.tile([CH, NCH * K], dt, tag="hall2", bufs=2,
                                     name=f"hall2_{b}")

                    def sink(ch, gta, gtb):
                        ps = cpsum.tile([CH, K], dt, tag="cK", bufs=2)
                        nc.tensor.matmul(out=ps[:], lhsT=gta[:], rhs=w2a_t[:],
                                         start=True, stop=False)
                        nc.tensor.matmul(out=ps[:], lhsT=gtb[:], rhs=w2b_t[:],
                                         start=False, stop=True)
                        nc.vector.tensor_copy(hall[:, ch * K:(ch + 1) * K],
                                              ps[:])

                    def finish():
                        tab_write(htab2[b][:], hall[:])
                    return sink, finish

                def conv3_sink(b):
                    def sink(ch, gta, gtb):
                        ps = cpsum.tile([K, CH], dt, tag="c3", bufs=2)
                        nc.tensor.matmul(out=ps[:], lhsT=w3a_t[:], rhs=gta[:],
                                         start=True, stop=False)
                        nc.tensor.matmul(out=ps[:], lhsT=w3b_t[:], rhs=gtb[:],
                                         start=False, stop=True)
                        fc = work.tile([K, CH], dt, tag="fc3")
                        nc.vector.tensor_copy(fc[:], ps[:])
                        nc.sync.dma_start(
                            bounce[:, :, b, ch * CH:(ch + 1) * CH], fc[:])

                    def finish():
                        pass
                    return sink, finish

                def pipe_once():
                    for b in range(BL):
                        if probe != "noconv1":
                            conv1(b)
                        s2, f2 = conv2_sink(b)
                        glayer(b, htab1[b], w2a_t, w2b_t, s2,
                               capture_ga=(dbg and b == 0))
                        f2()
                        s3, f3 = conv3_sink(b)
                        glayer(b, htab2[b], w3a_t, w3b_t, s3)
                        f3()
                        if dbg and b == 0:
                            nc.sync.dma_start(dbg_h1[:], htab1[0][:])
                            nc.sync.dma_start(dbg_h2[:], htab2[0][:])

                    tc.strict_bb_all_engine_barrier()

                    # ---- fc1 (contraction-parallel) ----
                    nc.gpsimd.collective_compute(
                        "AllToAll", mybir.AluOpType.bypass, replica_groups=rg,
                        ins=[bounce.opt()], outs=[recv.opt()])

                pipe_reps = reps if sect == "pipe" else 1
                for rep in range(pipe_reps):
                    pipe_once()
                if sect == "a2a":
                    for rep in range(reps - 1):
                        nc.gpsimd.collective_compute(
                            "AllToAll", mybir.AluOpType.bypass,
                            replica_groups=rg,
                            ins=[bounce.opt()], outs=[recv.opt()])
                if dbg:
                    nc.sync.dma_start(dbg_bnc[:], bounce[:])
                    nc.sync.dma_start(dbg_rcv[:], recv[:])

                fc1_reps = reps if sect == "fc1" else 1
                SEG = FP // 4
                with tc.tile_pool(name="fpsum", bufs=1, space="PSUM") as fpsum:
                    for rep in range(fc1_reps):
                        y1ps = fpsum.tile([H1, B], dt, tag="y1")
                        nst = KL * NCH
                        st = 0
                        for kl in range(KL):
                            for seg in range(FP // SEG):
                                rt = work.tile([B, SEG], dt, tag="rt", bufs=2)
                                nc.sync.dma_start(
                                    rt[:],
                                    recv[:, kl, :, seg * SEG:(seg + 1) * SEG])
                                for ch in range(SEG // CH):
                                    pst = cpsum.tile([CH, B], dt, tag="tT",
                                                     bufs=3)
                                    nc.tensor.transpose(
                                        pst[:], rt[:, ch * CH:(ch + 1) * CH],
                                        ident[:B, :B])
                                    ltt = work.tile([CH, B], dth, tag="ltt")
                                    nc.vector.tensor_copy(ltt[:], pst[:])
                                    wt = work.tile([CH, H1], dth, tag="fw")
                                    r0 = kl * FP + seg * SEG + ch * CH
                                    nc.sync.dma_start(
                                        wt[:], fc1wt[r0:r0 + CH, :])
                                    nc.tensor.matmul(
                                        out=y1ps[:], lhsT=wt[:], rhs=ltt[:],
                                        start=(st == 0), stop=(st == nst - 1))
                                    st += 1
                        y1l = work.tile([H1, B], dt, tag="y1l")
                        nc.vector.tensor_copy(y1l[:], y1ps[:])
                        nc.sync.dma_start(y1snd[:], y1l[:])

                nc.gpsimd.collective_compute(
                    "AllReduce", mybir.AluOpType.add, replica_groups=rg,
                    ins=[y1snd.opt()], outs=[y1rcv.opt()])
                if dbg:
                    nc.sync.dma_start(dbg_y1[:], y1rcv[:])

                # ---- head (replicated) ----
                def bn_relu(y, h, g_ap, b_ap, relu=True):
                    """In-place batchnorm(+relu) on SBUF tile y [h, B]."""
                    mean = work.tile([h, 1], dt, tag=f"bn_m{h}")
                    nc.vector.reduce_sum(mean[:], y[:],
                                         axis=mybir.AxisListType.X)
                    nc.vector.tensor_scalar_mul(mean[:], mean[:], 1.0 / B)
                    sq = work.tile([h, B], dt, tag=f"bn_sq{h}")
                    nc.vector.tensor_tensor(out=sq[:], in0=y[:], in1=y[:],
                                            op=mybir.AluOpType.mult)
                    var = work.tile([h, 1], dt, tag=f"bn_v{h}")
                    nc.vector.reduce_sum(var[:], sq[:],
                                         axis=mybir.AxisListType.X)
                    nc.vector.tensor_scalar_mul(var[:], var[:], 1.0 / B)
                    m2 = work.tile([h, 1], dt, tag=f"bn_m2{h}")
                    nc.vector.tensor_tensor(out=m2[:], in0=mean[:],
                                            in1=mean[:],
                                            op=mybir.AluOpType.mult)
                    nc.vector.tensor_tensor(out=var[:], in0=var[:], in1=m2[:],
                                            op=mybir.AluOpType.subtract)
                    nc.vector.tensor_scalar_add(var[:], var[:], cfg.EPS)
                    std = work.tile([h, 1], dt, tag=f"bn_s{h}")
                    nc.scalar.activation(std[:], var[:],
                                         mybir.ActivationFunctionType.Sqrt,
                                         bias=zcol[:h, :1])
                    rstd = work.tile([h, 1], dt, tag=f"bn_r{h}")
                    nc.vector.reciprocal(rstd[:], std[:])
                    gl = work.tile([h, 1], dt, tag=f"bn_g{h}")
                    nc.sync.dma_start(gl[:], g_ap[:])
                    bl = work.tile([h, 1], dt, tag=f"bn_b{h}")
                    nc.sync.dma_start(bl[:], b_ap[:])
                    scale = work.tile([h, 1], dt, tag=f"bn_sc{h}")
                    nc.vector.tensor_tensor(out=scale[:], in0=rstd[:],
                                            in1=gl[:],
                                            op=mybir.AluOpType.mult)
                    shift = work.tile([h, 1], dt, tag=f"bn_sh{h}")
                    nc.vector.tensor_tensor(out=shift[:], in0=mean[:],
                                            in1=scale[:],
                                            op=mybir.AluOpType.mult)
                    nc.vector.tensor_tensor(out=shift[:], in0=bl[:],
                                            in1=shift[:],
                                            op=mybir.AluOpType.subtract)
                    nc.vector.tensor_scalar(
                        out=y[:], in0=y[:], scalar1=scale[:], scalar2=shift[:],
                        op0=mybir.AluOpType.mult, op1=mybir.AluOpType.add)
                    if relu:
                        nc.scalar.activation(y[:], y[:],
                                             mybir.ActivationFunctionType.Relu,
                                             bias=zcol[:h, :1])

                y1 = work.tile([H1, B], dt, tag="y1h")
                nc.sync.dma_start(y1[:], y1rcv[:])
                f1b = work.tile([H1, 1], dt, tag="f1b")
                nc.sync.dma_start(f1b[:], fc1b[:])
                nc.vector.tensor_scalar_add(y1[:], y1[:], f1b[:])
                bn_relu(y1, H1, bn1g, bn1b)

                w2f = work.tile([H1, H2], dt, tag="w2f")
                nc.sync.dma_start(w2f[:], fc2wt[:])
                ps2 = cpsum.tile([H2, B], dt, tag="c3", bufs=2)
                nc.tensor.matmul(out=ps2[:], lhsT=w2f[:], rhs=y1[:],
                                 start=True, stop=True)
                y2 = work.tile([H2, B], dt, tag="y2h")
                nc.vector.tensor_copy(y2[:], ps2[:])
                f2b = work.tile([H2, 1], dt, tag="f2b")
                nc.sync.dma_start(f2b[:], fc2b[:])
                nc.vector.tensor_scalar_add(y2[:], y2[:], f2b[:])
                bn_relu(y2, H2, bn2g, bn2b)

                wof = work.tile([H2, NCLS], dt, tag="wof")
                nc.sync.dma_start(wof[:], fcowt[:])
                pso = cpsum.tile([NCLS, B], dt, tag="c3", bufs=2)
                nc.tensor.matmul(out=pso[:], lhsT=wof[:], rhs=y2[:],
                                 start=True, stop=True)
                yo = work.tile([NCLS, B], dt, tag="yo")
                nc.vector.tensor_copy(yo[:], pso[:])
                fob = work.tile([NCLS, 1], dt, tag="fob")
                nc.sync.dma_start(fob[:], fcob[:])
                nc.vector.tensor_scalar_add(yo[:], yo[:], fob[:])
                nc.sync.dma_start(out[:], yo[:])

    nc.compile()
    return nc


_CACHE: dict = {}


def _get_program(cfg: Cfg):
    key = cfg
    if key not in _CACHE:
        _CACHE[key] = build_program(cfg)
    return _CACHE[key]


def kernel(**inputs) -> np.ndarray:
    from concourse import bass_utils

    cfg = CFG
    nc = _get_program(cfg)
    in_maps = prep_core_inputs(cfg, **inputs)
    res = bass_utils.run_bass_kernel_spmd(
        nc, in_maps, core_ids=list(range(cfg.ncores)))
    return np.ascontiguousarray(res.results[0]["out"].T)

